# revision 1
# baseline (speedup 1.0000x reference)
"""Multi-head self-attention (full-embed, no head split) on 8 Trainium2 cores.

Sharding: data-parallel over (batch=4) x (query-half=2) = 8 cores.
Each core computes V for the full 2048-row sequence of its batch (duplicated
across the core pair), attention scores for its 1024 query rows, softmax,
weights @ V, and the output projection for its rows.

All device matmuls run as float32r (TF32-like, full PE rate); accumulation is
fp32. Key algebraic folds (all host-side, weight-only precomputes):
  - Q and K only enter via Q.K^T, so the two projections are fused into one
    matrix M = Wk^T @ Wq (host, fp64). On device: D = M-matmul of x^T, then
    scores^T = X @ D. This removes the Q and K projections entirely.
  - Score bias terms: the q-dependent parts cancel in softmax; the
    k-dependent part is sum_e X[k,e]*a2[e] with a2 = Wk^T @ bq, which folds
    into D's per-partition bias during the PSUM->SBUF copy.
  - V and Wo only appear as (softmax.V).Wo^T = softmax.(X @ (Wo@Wv)^T), so
    they are fused into one host matrix G = Wo @ Wv; the device projects
    U = X @ G^T once and the AV matmul directly yields the final output
    (normalize + bias fused into its PSUM drain). This removes the output
    projection entirely. The V bias folds into the output bias
    (bo' = bo + Wo @ bv, exact since softmax weights sum to 1).
Layout choices:
  - x is passed transposed per core as xt [E, S], with the core's query half
    permuted to the front (softmax over k is permutation-invariant as long as
    V uses the same k order, which it does); xt stays resident and serves as
    the stationary operand for both V-projection and the scores matmul.
  - scores^T is [k, q] so AV needs no transpose of the softmax weights; the
    softmax denominator Z is a [1, q] row via a ones-column matmul, broadcast
    to [128, q] via a K=1 ones-row matmul.
  - U (natural [s, f] layout): the f<512 half stays resident in SBUF; only
    the f>=512 half is spilled to DRAM and streamed back during AV in
    [128, 512] chunks.
  - The output projection is computed transposed (out^T [f, q]) so its weight
    tiles stream as small slices and its bias is per-partition; the host
    transposes the result back.
"""
import sys

sys.path.insert(0, '/opt/trn_rl_repo')

import numpy as np

import concourse.bass as bass
import concourse.bacc as bacc
import concourse.tile as tile
import concourse.mybir as mybir
from concourse import bass_utils

F32 = mybir.dt.float32
F32R = mybir.dt.float32r
AF = mybir.ActivationFunctionType

N_CORES = 8
B, S, E = 4, 2048, 1024
SH = S // 2          # per-core query rows
P = 128
EO = E // P          # 8 contraction chunks
FO = E // P          # 8 output-feature chunks
KO = S // P          # 16 key chunks
QB = 512             # q block (PSUM free dim)
NQB = SH // QB       # 2 q blocks per core
SCALE = 1.0 / np.sqrt(np.float32(E))

_CACHE = {}


def build_nc(loop_iters=None, stagger=True, vch_bufs=6):
    """Build + compile the Bass module. loop_iters wraps the whole body in a
    hardware loop (used only for timing amplification by test harnesses)."""
    nc = bacc.Bacc("TRN2", target_bir_lowering=False, debug=False,
                   enable_asserts=False, num_devices=N_CORES)

    xt_ap = nc.dram_tensor("xt", [E, S], F32R, kind="ExternalInput").ap()
    mt_ap = nc.dram_tensor("mt", [EO, P, EO, P], F32R, kind="ExternalInput").ap()
    wv_ap = nc.dram_tensor("wv", [2, P, EO, 512], F32R, kind="ExternalInput").ap()
    a2r_ap = nc.dram_tensor("a2r", [P, EO], F32, kind="ExternalInput").ap()
    bor_ap = nc.dram_tensor("bor", [P, FO], F32, kind="ExternalInput").ap()
    ones_ap = nc.dram_tensor("ones", [P, P], F32R, kind="ExternalInput").ap()
    # transposed output; host transposes back
    out_ap = nc.dram_tensor("out", [E, SH], F32, kind="ExternalOutput").ap()

    with tile.TileContext(nc) as tc:
        persist = tc.alloc_tile_pool(name="persist", bufs=1)
        dramp = tc.alloc_tile_pool(name="dramp", bufs=1, space="DRAM")

        def body():
            xt_sb = persist.tile([P, EO, S], F32R, name="xt_sb")
            ones_sb = persist.tile([P, P], F32R, name="ones_sb")
            u0_sb = persist.tile([P, KO, 512], F32R, name="u0_sb")
            v_dram = dramp.tile([KO, P, 512], F32R, name="v_dram")

            psB = tc.alloc_tile_pool(name="psB", bufs=1, space="PSUM")

            # blk_b is allocated before vpool (they coexist) so block-0's
            # score-weight slices can be queued at the head of the DMA queue
            # and D(0) never waits on the V-phase DMA tail.
            blk_b = tc.alloc_tile_pool(name="blk_b", bufs=1)

            # ---- Phase 1: V (natural [s, f], bias-free) -> DRAM spill.
            # wv half 0 is queued before the 8MB xt transfer so the first
            # matmuls only wait on xt chunk 0.
            vpool = tc.alloc_tile_pool(name="vpool", bufs=1)
            wv_next = vpool.tile([P, EO, 512], F32R, tag="wv", bufs=2,
                                 name="wv_t")
            nc.sync.dma_start(out=wv_next, in_=wv_ap[0])
            mt_first = blk_b.tile([P, EO, P], F32R, tag="mt", bufs=2,
                                  name="mt_t")
            nc.sync.dma_start(out=mt_first, in_=mt_ap[0])
            a2r_sb = blk_b.tile([P, EO], F32, tag="a2r", bufs=1, name="a2r_sb")
            nc.sync.dma_start(out=a2r_sb, in_=a2r_ap)
            bor_sb = blk_b.tile([P, FO], F32, tag="bor", bufs=1, name="bor_sb")
            nc.sync.dma_start(out=bor_sb, in_=bor_ap)
            for eo in range(EO):
                nc.sync.dma_start(out=xt_sb[:, eo, :],
                                  in_=xt_ap[eo * P:(eo + 1) * P, :])
            nc.sync.dma_start(out=ones_sb, in_=ones_ap)

            for ft in range(2):
                wv_t = wv_next
                if ft + 1 < 2:
                    wv_next = vpool.tile([P, EO, 512], F32R, tag="wv", bufs=2,
                                         name="wv_t")
                    nc.sync.dma_start(out=wv_next, in_=wv_ap[ft + 1])
                for so in range(KO):
                    psv = psB.tile([P, 512], F32, tag="psv", bufs=2, name="psv")
                    for eo in range(EO):
                        nc.tensor.matmul(psv, lhsT=xt_sb[:, eo, so * P:(so + 1) * P],
                                         rhs=wv_t[:, eo, :],
                                         start=(eo == 0), stop=(eo == EO - 1))
                    if ft == 0:
                        # f<512 half of U stays resident in SBUF
                        with nc.allow_low_precision(
                                reason="U feeds fp32r AU matmul"):
                            nc.vector.tensor_copy(out=u0_sb[:, so, :], in_=psv)
                    else:
                        vst = vpool.tile([P, 512], F32R, tag="vst", bufs=2,
                                         name="vst")
                        with nc.allow_low_precision(
                                reason="U feeds fp32r AU matmul"):
                            nc.vector.tensor_copy(out=vst, in_=psv)
                        nc.sync.dma_start(out=v_dram[so], in_=vst)
            vpool.release()
            psB.release()

            # ---- Phase 2: per q-block attention + output projection.
            # ps_mid is allocated first so it (not ps_sc) lands on the V-phase
            # psum banks: its first use (Z) is late, while ps_sc (D/scores)
            # starts immediately after the V matmuls.
            ps_mid = tc.alloc_tile_pool(name="ps_mid", bufs=1, space="PSUM")
            ps_sc = tc.alloc_tile_pool(name="ps_sc", bufs=1, space="PSUM")
            blk = tc.alloc_tile_pool(name="blk", bufs=1)

            def emit_scores(qb, mt0=None):
                """D = M . x^T (+ a2 bias) -> scores^T -> exp -> Z -> zinv
                -> broadcast."""
                q0 = qb * QB
                exp_sb = blk.tile([P, KO, QB], F32R, tag="exp", bufs=1,
                                  name="exp_sb")
                d_sb = blk.tile([P, EO, QB], F32R, tag="d", bufs=1, name="d_sb")
                if mt0 is not None:
                    mt_next = mt0
                else:
                    mt_next = blk_b.tile([P, EO, P], F32R, tag="mt", bufs=2,
                                         name="mt_t")
                    nc.sync.dma_start(out=mt_next, in_=mt_ap[0])
                for eod in range(EO):
                    mt_t = mt_next
                    if eod + 1 < EO:
                        mt_next = blk_b.tile([P, EO, P], F32R, tag="mt", bufs=2,
                                             name="mt_t")
                        nc.sync.dma_start(out=mt_next, in_=mt_ap[eod + 1])
                    psd = ps_sc.tile([P, QB], F32, tag="pss", bufs=2, name="psd")
                    for eo in range(EO):
                        nc.tensor.matmul(psd, lhsT=mt_t[:, eo, :],
                                         rhs=xt_sb[:, eo, q0:q0 + QB],
                                         start=(eo == 0), stop=(eo == EO - 1))
                    nc.scalar.activation(out=d_sb[:, eod, :], in_=psd,
                                         func=AF.Identity,
                                         bias=a2r_sb[:, eod:eod + 1], scale=1.0)

                for ko in range(KO):
                    pss = ps_sc.tile([P, QB], F32, tag="pss", bufs=2, name="pss")
                    for eo in range(EO):
                        nc.tensor.matmul(pss, lhsT=xt_sb[:, eo, ko * P:(ko + 1) * P],
                                         rhs=d_sb[:, eo, :],
                                         start=(eo == 0), stop=(eo == EO - 1))
                    nc.scalar.activation(out=exp_sb[:, ko, :], in_=pss,
                                         func=AF.Exp, scale=float(SCALE))
                psz = ps_mid.tile([P, QB], F32, tag="psa", bufs=6, name="psz")
                for ko in range(KO):
                    nc.tensor.matmul(psz[:1, :], lhsT=ones_sb[:, 0:1],
                                     rhs=exp_sb[:, ko, :],
                                     start=(ko == 0), stop=(ko == KO - 1))
                zinv = blk.tile([1, QB], F32R, tag="zinv", bufs=1, name="zinv")
                with nc.allow_low_precision(reason="zinv feeds fp32r matmul"):
                    nc.vector.reciprocal(out=zinv[:1, :], in_=psz[:1, :])
                psb = ps_sc.tile([P, QB], F32, tag="pss", bufs=2, name="psb")
                nc.tensor.matmul(psb, lhsT=ones_sb[:1, :], rhs=zinv[:1, :],
                                 start=True, stop=True)
                zb_sb = blk.tile([P, QB], F32, tag="zb", bufs=1, name="zb_sb")
                nc.vector.tensor_copy(out=zb_sb, in_=psb)
                return exp_sb, zb_sb

            def emit_au_out(qb, exp_sb, zb_sb):
                """out^T[f, q] = (sum_k U[k, f] * exp[k, q]) * zinv[q] + bo'"""
                q0 = qb * QB
                for ft in range(2):
                    psp = [ps_mid.tile([P, QB], F32, tag="psa", bufs=6,
                                       name=f"psa{j}") for j in range(4)]
                    for ko in range(KO):
                        if ft == 0:
                            usrc = u0_sb[:, ko, :]
                        else:
                            vch = blk.tile([P, 512], F32R, tag="vch",
                                           bufs=vch_bufs, name="vch")
                            nc.sync.dma_start(out=vch, in_=v_dram[ko])
                            usrc = vch
                        for j in range(4):
                            nc.tensor.matmul(psp[j],
                                             lhsT=usrc[:, j * P:(j + 1) * P],
                                             rhs=exp_sb[:, ko, :],
                                             start=(ko == 0),
                                             stop=(ko == KO - 1))
                    for j in range(4):
                        fo = ft * 4 + j
                        osa = blk_b.tile([P, QB], F32, tag="osa", bufs=2,
                                         name="osa")
                        nc.vector.tensor_mul(out=osa, in0=psp[j], in1=zb_sb)
                        ost = blk_b.tile([P, QB], F32, tag="ost", bufs=2,
                                         name="ost")
                        nc.scalar.activation(out=ost, in_=osa, func=AF.Identity,
                                             bias=bor_sb[:, fo:fo + 1], scale=1.0)
                        nc.sync.dma_start(
                            out=out_ap[fo * P:(fo + 1) * P, q0:q0 + QB], in_=ost)

            for qb in range(NQB):
                expq, zbq = emit_scores(qb, mt0=mt_first if qb == 0 else None)
                emit_au_out(qb, expq, zbq)

            blk.release()
            blk_b.release()
            ps_sc.release()
            ps_mid.release()

        if loop_iters is None:
            body()
        else:
            with tc.For_i(0, loop_iters):
                body()

        dramp.release()
        persist.release()

    nc.compile()
    return nc


def _prep_shared(Wq, bq, Wk, bk, Wv, bv, Wo, bo):
    def chunk_w(W, free):
        wT = np.ascontiguousarray(np.asarray(W, dtype=np.float32).T)
        n = E // free
        return np.ascontiguousarray(
            wT.reshape(EO, P, n, free).transpose(2, 1, 0, 3))

    W64 = {k: np.asarray(v, dtype=np.float64)
           for k, v in dict(Wq=Wq, bq=bq, Wk=Wk, Wv=Wv, bv=bv, Wo=Wo,
                            bo=bo).items()}
    # Q.K^T and V.Wo^T weight fusions (see module docstring)
    M = (W64["Wk"].T @ W64["Wq"]).astype(np.float32)      # [e, e']
    G = (W64["Wo"] @ W64["Wv"]).astype(np.float32)        # [f, e']
    a2 = (W64["Wk"].T @ W64["bq"]).astype(np.float32)     # [e]
    bo_folded = (W64["bo"] + W64["Wo"] @ W64["bv"]).astype(np.float32)
    return {
        "mt": chunk_w(M, P),
        "wv": chunk_w(G, 512),
        "a2r": np.ascontiguousarray(a2.reshape(EO, P).T),
        "bor": np.ascontiguousarray(bo_folded.reshape(FO, P).T),
        "ones": np.ones((P, P), dtype=np.float32),
    }


def make_in_maps(x, Wq, bq, Wk, bk, Wv, bv, Wo, bo):
    shared = _prep_shared(Wq, bq, Wk, bk, Wv, bv, Wo, bo)
    in_maps = []
    for c in range(N_CORES):
        b, h = c // 2, c % 2
        xt = np.asarray(x[b]).T  # [E, S]
        if h == 0:
            xt_p = np.ascontiguousarray(xt)
        else:
            xt_p = np.ascontiguousarray(
                np.concatenate([xt[:, SH:], xt[:, :SH]], axis=1))
        m = {"xt": xt_p}
        m.update(shared)
        in_maps.append(m)
    return in_maps


def kernel(x, Wq, bq, Wk, bk, Wv, bv, Wo, bo):
    x = np.asarray(x, dtype=np.float32)
    args = [np.asarray(a, dtype=np.float32)
            for a in (Wq, bq, Wk, bk, Wv, bv, Wo, bo)]
    if "nc" not in _CACHE:
        _CACHE["nc"] = build_nc()
    nc = _CACHE["nc"]
    in_maps = make_in_maps(x, *args)
    res = bass_utils.run_bass_kernel_spmd(nc, in_maps,
                                          core_ids=list(range(N_CORES)))
    out = np.empty((B, S, E), dtype=np.float32)
    for c in range(N_CORES):
        b, h = c // 2, c % 2
        out[b, h * SH:(h + 1) * SH, :] = res.results[c]["out"].T
    return out



# revision 2
# speedup vs baseline: 5.3656x; 5.3656x over previous
"""Multi-head self-attention (full-embed, no head split) on 8 Trainium2 cores.

Sharding: data-parallel over (batch=4) x (query-half=2) = 8 cores.
Each core computes U for the full 2048-row sequence of its batch (duplicated
across the core pair), attention scores for its 1024 query rows, softmax,
weights @ U, and the (folded) output projection for its rows.

Weight folds (host-side, fp64):
  - Q/K projections fused into M = Wk^T @ Wq; on device D = M . x^T, then
    scores^T = X @ D.  Score bias: q-dependent parts cancel in softmax; the
    k-part folds into D's per-partition bias (a2 = Wk^T @ bq).
  - V and out_proj fused into G = Wo @ Wv; U = X @ G^T so the AV matmul
    directly yields the final output.  V bias folds into the output bias
    (bo' = bo + Wo @ bv, exact since softmax weights sum to 1).

v2 layout/engine choices (vs the fp32r/DRAM-spill v1):
  - All matmul operands are bf16: halves DMA traffic and SBUF footprint so
    EVERYTHING stays resident: xt 4MB (2 half-chunks per eo so phase 1 can
    start after one quarter lands), mt 2MB (loaded once, reused by both q
    blocks), wv 2MB, U 4MB (no DRAM spill / AV reload).  Accumulation in
    fp32 PSUM throughout; rel-err budget 2e-2, measured ~2e-3.
  - Softmax denominator Z: a DVE running-sum over the 16 exp chunks (f32),
    then ONE ones-column matmul instead of 16 (saves ~15k PE rows/block).
    The psz and zinv-broadcast matmuls are emitted INSIDE the AV matmul
    stream so the PE never waits on the exp->acc->reciprocal chain.
  - Phase 1 runs eo-major over so-groups of 4 with PSUM bank ping-pong
    (bufs=8): the first matmul needs only xt chunk (eo=0, q-quarter 0), and
    group transitions never wait on PSUM drains.
  - Loads go on the SP HWDGE queue, output stores on the Activation queue:
    stores never head-of-line-block the next iteration's xt prefetch (the
    timing loop, and any back-to-back use, overlaps iteration i+1's phase-1
    loads with iteration i's AV tail).
  - Output is computed transposed (out^T [f, q], per-partition bias) and
    transposed back on host.
"""
import sys

sys.path.insert(0, '/opt/trn_rl_repo')

import numpy as np
import ml_dtypes

import concourse.bass as bass
import concourse.bacc as bacc
import concourse.tile as tile
import concourse.mybir as mybir
from concourse import bass_utils

F32 = mybir.dt.float32
F32R = mybir.dt.float32r
BF16 = mybir.dt.bfloat16
AF = mybir.ActivationFunctionType
BFNP = ml_dtypes.bfloat16

N_CORES = 8
B, S, E = 4, 2048, 1024
SH = S // 2          # per-core query rows
P = 128
EO = E // P          # 8 contraction chunks
FO = E // P          # 8 output-feature chunks
KO = S // P          # 16 key chunks
QB = 512             # q block (PSUM free dim)
NQB = SH // QB       # 2 q blocks per core
SCALE = 1.0 / np.sqrt(np.float32(E))

_CACHE = {}


def build_nc(loop_iters=None):
    """Build + compile the Bass module. loop_iters wraps the whole body in a
    hardware loop (used only for timing amplification by test harnesses)."""
    nc = bacc.Bacc("TRN2", target_bir_lowering=False, debug=False,
                   enable_asserts=False, num_devices=N_CORES)

    xt_ap = nc.dram_tensor("xt", [E, S], BF16, kind="ExternalInput").ap()
    mt_ap = nc.dram_tensor("mt", [EO, P, EO, P], BF16, kind="ExternalInput").ap()
    wv_ap = nc.dram_tensor("wv", [2, P, EO, 512], BF16, kind="ExternalInput").ap()
    a2r_ap = nc.dram_tensor("a2r", [P, EO], F32, kind="ExternalInput").ap()
    bor_ap = nc.dram_tensor("bor", [P, FO], F32, kind="ExternalInput").ap()
    ones_ap = nc.dram_tensor("ones", [P, P], F32R, kind="ExternalInput").ap()
    # transposed output; host transposes back
    out_ap = nc.dram_tensor("out", [E, SH], F32, kind="ExternalOutput").ap()

    with tile.TileContext(nc) as tc:
        persist = tc.alloc_tile_pool(name="persist", bufs=1)

        def body():
            xt_sb = persist.tile([P, EO, S], BF16, name="xt_sb")
            mt_sb = persist.tile([P, EO, EO, P], BF16, name="mt_sb")
            wv_sb = persist.tile([P, 2, EO, 512], BF16, name="wv_sb")
            u_sb = persist.tile([P, KO, E], BF16, name="u_sb")
            ones_sb = persist.tile([P, P], F32R, name="ones_sb")
            a2r_sb = persist.tile([P, EO], F32, name="a2r_sb")
            bor_sb = persist.tile([P, FO], F32, name="bor_sb")

            # ---- loads (SP queue), in phase-1 consumption order.
            # xt in quarter-chunks so the first V matmul waits on 256KB, not
            # 4MB; wv ft=0 chunk-interleaved for the same reason.
            for eo in range(EO):
                nc.sync.dma_start(out=wv_sb[:, 0, eo, :], in_=wv_ap[0, :, eo, :])
                nc.sync.dma_start(out=xt_sb[:, eo, 0:512],
                                  in_=xt_ap[eo * P:(eo + 1) * P, 0:512])
            for q4 in range(1, 4):
                for eo in range(EO):
                    nc.sync.dma_start(
                        out=xt_sb[:, eo, q4 * 512:(q4 + 1) * 512],
                        in_=xt_ap[eo * P:(eo + 1) * P, q4 * 512:(q4 + 1) * 512])
            nc.sync.dma_start(out=wv_sb[:, 1], in_=wv_ap[1])
            for eod in range(EO):
                nc.sync.dma_start(out=mt_sb[:, eod], in_=mt_ap[eod])
            nc.sync.dma_start(out=a2r_sb, in_=a2r_ap)
            nc.sync.dma_start(out=bor_sb, in_=bor_ap)
            nc.sync.dma_start(out=ones_sb, in_=ones_ap)

            # ---- Phase 1: U = X @ G^T (natural [s, f]), fully resident.
            psB = tc.alloc_tile_pool(name="psB", bufs=1, space="PSUM")
            for ft in range(2):
                for grp in range(4):
                    psv = [psB.tile([P, 512], F32, tag="psv", bufs=8,
                                    name=f"psv{j}") for j in range(4)]
                    for eo in range(EO):
                        for j in range(4):
                            so = grp * 4 + j
                            nc.tensor.matmul(
                                psv[j],
                                lhsT=xt_sb[:, eo, so * P:(so + 1) * P],
                                rhs=wv_sb[:, ft, eo, :],
                                start=(eo == 0), stop=(eo == EO - 1))
                    for j in range(4):
                        so = grp * 4 + j
                        dst = u_sb[:, so, ft * 512:(ft + 1) * 512]
                        with nc.allow_low_precision(
                                reason="U feeds bf16 AV matmul"):
                            if j % 2 == 0:
                                nc.scalar.activation(out=dst, in_=psv[j],
                                                     func=AF.Identity, scale=1.0)
                            else:
                                nc.vector.tensor_copy(out=dst, in_=psv[j])
            psB.release()

            # ---- Phase 2: per q-block attention + folded output projection.
            ps_mid = tc.alloc_tile_pool(name="ps_mid", bufs=1, space="PSUM")
            ps_sc = tc.alloc_tile_pool(name="ps_sc", bufs=1, space="PSUM")
            blk = tc.alloc_tile_pool(name="blk", bufs=1)

            for qb in range(NQB):
                q0 = qb * QB
                exp_sb = blk.tile([P, KO, QB], BF16, tag="exp", bufs=1,
                                  name="exp_sb")
                d_sb = blk.tile([P, EO, QB], BF16, tag="d", bufs=1, name="d_sb")
                acc = blk.tile([P, QB], F32R, tag="acc", bufs=1, name="acc")

                # D = M . x^T (+ a2 per-partition bias)
                for eod in range(EO):
                    psd = ps_sc.tile([P, QB], F32, tag="pss", bufs=2, name="psd")
                    for eo in range(EO):
                        nc.tensor.matmul(psd, lhsT=mt_sb[:, eod, eo, :],
                                         rhs=xt_sb[:, eo, q0:q0 + QB],
                                         start=(eo == 0), stop=(eo == EO - 1))
                    with nc.allow_low_precision(reason="D feeds bf16 matmul"):
                        nc.scalar.activation(out=d_sb[:, eod, :], in_=psd,
                                             func=AF.Identity,
                                             bias=a2r_sb[:, eod:eod + 1],
                                             scale=1.0)

                # scores^T = X @ D -> exp (bf16) + DVE running sum (f32)
                for ko in range(KO):
                    pss = ps_sc.tile([P, QB], F32, tag="pss", bufs=2, name="pss")
                    for eo in range(EO):
                        nc.tensor.matmul(pss,
                                         lhsT=xt_sb[:, eo, ko * P:(ko + 1) * P],
                                         rhs=d_sb[:, eo, :],
                                         start=(eo == 0), stop=(eo == EO - 1))
                    with nc.allow_low_precision(
                            reason="softmax weights feed bf16 AV matmul"):
                        nc.scalar.activation(out=exp_sb[:, ko, :], in_=pss,
                                             func=AF.Exp, scale=float(SCALE))
                    with nc.allow_low_precision(reason="Z accum is fp32 bits"):
                        if ko == 0:
                            nc.vector.tensor_copy(out=acc, in_=exp_sb[:, 0, :])
                        else:
                            nc.vector.tensor_add(out=acc, in0=acc,
                                                 in1=exp_sb[:, ko, :])

                # AV + folded out-proj, with the Z matmuls slotted into the
                # AV stream (PE reaches them well after the DVE chain ends).
                zinv = blk.tile([1, QB], F32R, tag="zinv", bufs=1, name="zinv")
                zb_sb = blk.tile([P, QB], F32, tag="zb", bufs=1, name="zb_sb")
                psz = ps_mid.tile([P, QB], F32, tag="psa", bufs=6, name="psz")
                for ft in range(2):
                    psp = [ps_mid.tile([P, QB], F32, tag="psa", bufs=6,
                                       name=f"psa{j}") for j in range(4)]
                    for ko in range(KO):
                        for j in range(4):
                            nc.tensor.matmul(
                                psp[j],
                                lhsT=u_sb[:, ko, ft * 512 + j * P:
                                          ft * 512 + (j + 1) * P],
                                rhs=exp_sb[:, ko, :],
                                start=(ko == 0), stop=(ko == KO - 1))
                        if ft == 0 and ko == 3:
                            nc.tensor.matmul(psz[:1, :], lhsT=ones_sb[:, 0:1],
                                             rhs=acc, start=True, stop=True)
                            with nc.allow_low_precision(
                                    reason="zinv feeds f32r matmul"):
                                nc.vector.reciprocal(out=zinv[:1, :],
                                                     in_=psz[:1, :])
                        if ft == 0 and ko == 6:
                            psb = ps_sc.tile([P, QB], F32, tag="pss", bufs=2,
                                             name="psb")
                            nc.tensor.matmul(psb, lhsT=ones_sb[:1, :],
                                             rhs=zinv[:1, :],
                                             start=True, stop=True)
                            nc.vector.tensor_copy(out=zb_sb, in_=psb)
                    for j in range(4):
                        fo = ft * 4 + j
                        osa = blk.tile([P, QB], F32, tag="osa", bufs=2,
                                       name="osa")
                        nc.vector.tensor_mul(out=osa, in0=psp[j], in1=zb_sb)
                        ost = blk.tile([P, QB], F32, tag="ost", bufs=2,
                                       name="ost")
                        nc.scalar.activation(out=ost, in_=osa, func=AF.Identity,
                                             bias=bor_sb[:, fo:fo + 1],
                                             scale=1.0)
                        # stores ride the Activation HWDGE queue
                        nc.scalar.dma_start(
                            out=out_ap[fo * P:(fo + 1) * P, q0:q0 + QB],
                            in_=ost)

            blk.release()
            ps_sc.release()
            ps_mid.release()

        if loop_iters is None:
            body()
        else:
            with tc.For_i(0, loop_iters):
                body()

        persist.release()

    nc.compile()
    return nc


def _prep_shared(Wq, bq, Wk, bk, Wv, bv, Wo, bo):
    def chunk_w(W, free):
        wT = np.ascontiguousarray(np.asarray(W, dtype=np.float32).T)
        n = E // free
        return np.ascontiguousarray(
            wT.reshape(EO, P, n, free).transpose(2, 1, 0, 3).astype(BFNP))

    W64 = {k: np.asarray(v, dtype=np.float64)
           for k, v in dict(Wq=Wq, bq=bq, Wk=Wk, Wv=Wv, bv=bv, Wo=Wo,
                            bo=bo).items()}
    # Q.K^T and V.Wo^T weight fusions (see module docstring)
    M = (W64["Wk"].T @ W64["Wq"]).astype(np.float32)      # [e, e']
    G = (W64["Wo"] @ W64["Wv"]).astype(np.float32)        # [f, e']
    a2 = (W64["Wk"].T @ W64["bq"]).astype(np.float32)     # [e]
    bo_folded = (W64["bo"] + W64["Wo"] @ W64["bv"]).astype(np.float32)
    return {
        "mt": chunk_w(M, P),
        "wv": chunk_w(G, 512),
        "a2r": np.ascontiguousarray(a2.reshape(EO, P).T),
        "bor": np.ascontiguousarray(bo_folded.reshape(FO, P).T),
        "ones": np.ones((P, P), dtype=np.float32),
    }


def make_in_maps(x, Wq, bq, Wk, bk, Wv, bv, Wo, bo):
    shared = _prep_shared(Wq, bq, Wk, bk, Wv, bv, Wo, bo)
    in_maps = []
    for c in range(N_CORES):
        b, h = c // 2, c % 2
        xt = np.asarray(x[b]).T  # [E, S]
        if h == 0:
            xt_p = np.ascontiguousarray(xt.astype(BFNP))
        else:
            xt_p = np.ascontiguousarray(
                np.concatenate([xt[:, SH:], xt[:, :SH]], axis=1).astype(BFNP))
        m = {"xt": xt_p}
        m.update(shared)
        in_maps.append(m)
    return in_maps


def kernel(x, Wq, bq, Wk, bk, Wv, bv, Wo, bo):
    x = np.asarray(x, dtype=np.float32)
    args = [np.asarray(a, dtype=np.float32)
            for a in (Wq, bq, Wk, bk, Wv, bv, Wo, bo)]
    if "nc" not in _CACHE:
        _CACHE["nc"] = build_nc()
    nc = _CACHE["nc"]
    in_maps = make_in_maps(x, *args)
    res = bass_utils.run_bass_kernel_spmd(nc, in_maps,
                                          core_ids=list(range(N_CORES)))
    out = np.empty((B, S, E), dtype=np.float32)
    for c in range(N_CORES):
        b, h = c // 2, c % 2
        out[b, h * SH:(h + 1) * SH, :] = res.results[c]["out"].T
    return out


# revision 16
# speedup vs baseline: 6.0402x; 1.1257x over previous
"""Multi-head self-attention (full-embed, no head split) on 8 Trainium2 cores.

Sharding: data-parallel over (batch=4) x (query-half=2) = 8 cores.
Each core computes U for the full 2048-row sequence of its batch (duplicated
across the core pair), attention scores for its 1024 query rows, softmax,
weights @ U, and the (folded) output projection for its rows.

Weight folds (host-side, fp64):
  - Q/K projections fused into M = Wk^T @ Wq; on device D = M . x^T, then
    scores^T = X @ D.  Score bias: q-dependent parts cancel in softmax; the
    k-part folds into D's per-partition bias (a2 = Wk^T @ bq).
  - V and out_proj fused into G = Wo @ Wv; U = X @ G^T so the AV matmul
    directly yields the final output.  V bias folds into the output bias
    (bo' = bo + Wo @ bv, exact since softmax weights sum to 1).

Precision/engine choices:
  - Matmul stages can run in fp8e4m3 with MatmulPerfMode.DoubleRow: operand
    APs are [128, 2, free] pairs of adjacent contraction chunks, which the
    existing [p, chunk, free] tile layouts provide by plain slicing.  This
    halves both the PE row count and the matmul instruction count (measured
    ~82ns fixed overhead per matmul instruction on HW).  Stage set is
    FP8_STAGES ("v" V-proj, "d" D, "s" scores, "a" AV); non-fp8 stages use
    bf16 (fp32 PSUM accumulation everywhere).
  - Everything stays resident in SBUF (no DRAM spill): xt, mt, wv, U, exp.
    Loads ride the SP HWDGE queue in consumption order; output stores ride
    the Activation queue so they never head-of-line-block the next
    iteration's prefetch (relevant for the For_i timing loop / back-to-back
    calls).
  - Softmax denominator: when AV is fp8, Z accumulates on the PE as
    ones-pair DoubleRow matmuls interleaved into the scores stream (2-chunk
    lag, no stall); otherwise a DVE running sum.  The reciprocal broadcast
    (zb) matmuls are emitted inside the AV stream so the PE never waits.
  - Output is computed transposed (out^T [f, q], per-partition bias) and
    transposed back on host.
"""
import sys

sys.path.insert(0, '/opt/trn_rl_repo')

import numpy as np
import ml_dtypes

import concourse.bass as bass
import concourse.bacc as bacc
import concourse.tile as tile
import concourse.mybir as mybir
from concourse import bass_utils

F32 = mybir.dt.float32
F32R = mybir.dt.float32r
BF16 = mybir.dt.bfloat16
FP8 = mybir.dt.float8e4
AF = mybir.ActivationFunctionType
DR = mybir.MatmulPerfMode.DoubleRow
BFNP = ml_dtypes.bfloat16
F8NP = ml_dtypes.float8_e4m3

N_CORES = 8
B, S, E = 4, 2048, 1024
SH = S // 2          # per-core query rows
P = 128
EO = E // P          # 8 contraction chunks
FO = E // P          # 8 output-feature chunks
KO = S // P          # 16 key chunks
QB = 512             # q block (PSUM free dim)
NQB = SH // QB       # 2 q blocks per core
SCALE = 1.0 / np.sqrt(np.float32(E))

# Which matmul stages run fp8e4m3 + DoubleRow.  Speed/accuracy measured on
# HW (gate: rel_err < 2e-2; errors are run-to-run deterministic):
#   ""     254.9us  1.79e-3   (all-bf16)
#   "a"    228.5us  1.83e-2   <- shipped: AV + Z in fp8
#   "v"    254.7us  1.69e-2   (no speed gain)
#   "s"    252.0us  1.31e-2   (no speed gain)
#   "vdsa" 152.1us  3.09e-2   (fails gate; fp8 stage errors RSS-combine)
FP8_STAGES = "a"
Z_BF16 = False       # debug: force bf16 DVE-chain Z even with fp8 AV

_CACHE = {}


def build_nc(loop_iters=None, stages="vdsa", fp8=FP8_STAGES):
    """Build + compile the Bass module. loop_iters wraps the whole body in a
    hardware loop (used only for timing amplification by test harnesses).
    stages: subset of "vdsa" for timing bisection.  fp8: subset of "vdsa"
    running fp8+DoubleRow."""
    nc = bacc.Bacc("TRN2", target_bir_lowering=False, debug=False,
                   enable_asserts=False, num_devices=N_CORES)

    need_xt16 = not ({"v", "d", "s"} <= set(fp8))
    need_xt8 = bool({"v", "d", "s"} & set(fp8))

    aps = {}
    if need_xt16:
        aps["xt"] = nc.dram_tensor("xt", [E, S], BF16, kind="ExternalInput").ap()
    if need_xt8:
        aps["xt8"] = nc.dram_tensor("xt8", [E, S], FP8, kind="ExternalInput").ap()
    aps["mt"] = nc.dram_tensor("mt", [EO, P, EO, P], FP8 if "d" in fp8 else BF16,
                               kind="ExternalInput").ap()
    aps["wv"] = nc.dram_tensor("wv", [2, P, EO, 512], FP8 if "v" in fp8 else BF16,
                               kind="ExternalInput").ap()
    a2r_ap = nc.dram_tensor("a2r", [P, EO], F32, kind="ExternalInput").ap()
    bor_ap = nc.dram_tensor("bor", [P, FO], F32, kind="ExternalInput").ap()
    ones_ap = nc.dram_tensor("ones", [P, P], F32R, kind="ExternalInput").ap()
    if "a" in fp8:
        ones8_ap = nc.dram_tensor("ones8", [P, 2, P], FP8, kind="ExternalInput").ap()
    # transposed output; host transposes back
    out_ap = nc.dram_tensor("out", [E, SH], F32, kind="ExternalOutput").ap()

    d_dt = FP8 if "s" in fp8 else BF16
    e_dt = FP8 if "a" in fp8 else BF16

    with tile.TileContext(nc) as tc:
        persist = tc.alloc_tile_pool(name="persist", bufs=1)

        def body():
            xt_sb = (persist.tile([P, EO, S], BF16, name="xt_sb")
                     if need_xt16 else None)
            xt8_sb = (persist.tile([P, EO, S], FP8, name="xt8_sb")
                      if need_xt8 else None)
            mt_sb = persist.tile([P, EO, EO, P], FP8 if "d" in fp8 else BF16,
                                 name="mt_sb")
            wv_sb = persist.tile([P, 2, EO, 512], FP8 if "v" in fp8 else BF16,
                                 name="wv_sb")
            u_sb = persist.tile([P, KO, E], e_dt, name="u_sb")
            ones_sb = persist.tile([P, P], F32R, name="ones_sb")
            ones8_sb = (persist.tile([P, 2, P], FP8, name="ones8_sb")
                        if "a" in fp8 else None)
            a2r_sb = persist.tile([P, EO], F32, name="a2r_sb")
            bor_sb = persist.tile([P, FO], F32, name="bor_sb")

            vx = xt8_sb if "v" in fp8 else xt_sb
            dx = xt8_sb if "d" in fp8 else xt_sb
            sx = xt8_sb if "s" in fp8 else xt_sb

            # ---- loads (SP queue) in phase-1 consumption order.
            def load_xt(dst, src):
                for eo in range(EO):
                    nc.sync.dma_start(out=dst[:, eo, 0:512],
                                      in_=src[eo * P:(eo + 1) * P, 0:512])
                for q4 in range(1, 4):
                    for eo in range(EO):
                        nc.sync.dma_start(
                            out=dst[:, eo, q4 * 512:(q4 + 1) * 512],
                            in_=src[eo * P:(eo + 1) * P,
                                    q4 * 512:(q4 + 1) * 512])

            for eo in range(EO):
                nc.sync.dma_start(out=wv_sb[:, 0, eo, :], in_=aps["wv"][0, :, eo, :])
            if need_xt8:
                load_xt(xt8_sb, aps["xt8"])
            if need_xt16:
                load_xt(xt_sb, aps["xt"])
            nc.sync.dma_start(out=wv_sb[:, 1], in_=aps["wv"][1])
            for eod in range(EO):
                nc.sync.dma_start(out=mt_sb[:, eod], in_=aps["mt"][eod])
            nc.sync.dma_start(out=a2r_sb, in_=a2r_ap)
            nc.sync.dma_start(out=bor_sb, in_=bor_ap)
            nc.sync.dma_start(out=ones_sb, in_=ones_ap)
            if "a" in fp8:
                nc.sync.dma_start(out=ones8_sb, in_=ones8_ap)

            # ---- Phase 1: U = X @ G^T (natural [s, f]), fully resident.
            psB = tc.alloc_tile_pool(name="psB", bufs=1, space="PSUM")
            for ft in range(2 if "v" in stages else 0):
                for grp in range(4):
                    psv = [psB.tile([P, 512], F32, tag="psv", bufs=8,
                                    name=f"psv{j}") for j in range(4)]
                    if "v" in fp8:
                        for ec in range(4):
                            for j in range(4):
                                so = grp * 4 + j
                                nc.tensor.matmul(
                                    psv[j],
                                    lhsT=vx[:, 2 * ec:2 * ec + 2,
                                            so * P:(so + 1) * P],
                                    rhs=wv_sb[:, ft, 2 * ec:2 * ec + 2, :],
                                    start=(ec == 0), stop=(ec == 3),
                                    perf_mode=DR)
                    else:
                        for eo in range(EO):
                            for j in range(4):
                                so = grp * 4 + j
                                nc.tensor.matmul(
                                    psv[j],
                                    lhsT=vx[:, eo, so * P:(so + 1) * P],
                                    rhs=wv_sb[:, ft, eo, :],
                                    start=(eo == 0), stop=(eo == EO - 1))
                    usc = 1.0 / 32 if "v" in fp8 else 1.0
                    for j in range(4):
                        so = grp * 4 + j
                        dst = u_sb[:, so, ft * 512:(ft + 1) * 512]
                        with nc.allow_low_precision(
                                reason="U feeds low-precision AV matmul"):
                            if "a" in fp8 or j % 2 == 0:
                                # fp8 stores must go via the Activation
                                # engine (DVE fp8 writes corrupt)
                                nc.scalar.activation(out=dst, in_=psv[j],
                                                     func=AF.Identity, scale=usc)
                            elif "v" in fp8:
                                nc.vector.tensor_scalar_mul(out=dst,
                                                            in0=psv[j],
                                                            scalar1=usc)
                            else:
                                nc.vector.tensor_copy(out=dst, in_=psv[j])
            psB.release()

            # ---- Phase 2: per q-block attention + folded output projection.
            ps_mid = tc.alloc_tile_pool(name="ps_mid", bufs=1, space="PSUM")
            ps_sc = tc.alloc_tile_pool(name="ps_sc", bufs=1, space="PSUM")
            blk = tc.alloc_tile_pool(name="blk", bufs=1)

            for qb in range(NQB):
                q0 = qb * QB
                exp_sb = blk.tile([P, KO, QB], e_dt, tag="exp", bufs=1,
                                  name="exp_sb")
                expz_sb = (blk.tile([P, KO, QB], BF16, tag="expz", bufs=1,
                                    name="expz_sb")
                           if (Z_BF16 and "a" in fp8) else None)
                d_sb = blk.tile([P, EO, QB], d_dt, tag="d", bufs=1, name="d_sb")
                acc = (None if ("a" in fp8 and not Z_BF16) else
                       blk.tile([P, QB], F32R, tag="acc", bufs=1, name="acc"))
                psz = ps_mid.tile([P, QB], F32, tag="psa", bufs=6, name="psz")

                # D = M . x^T (+ a2 per-partition bias)
                for eod in range(EO if "d" in stages else 0):
                    psd = ps_sc.tile([P, QB], F32, tag="pss", bufs=2, name="psd")
                    if "d" in fp8:
                        for ec in range(4):
                            nc.tensor.matmul(
                                psd, lhsT=mt_sb[:, eod, 2 * ec:2 * ec + 2, :],
                                rhs=dx[:, 2 * ec:2 * ec + 2, q0:q0 + QB],
                                start=(ec == 0), stop=(ec == 3), perf_mode=DR)
                    else:
                        for eo in range(EO):
                            nc.tensor.matmul(psd, lhsT=mt_sb[:, eod, eo, :],
                                             rhs=dx[:, eo, q0:q0 + QB],
                                             start=(eo == 0), stop=(eo == EO - 1))
                    with nc.allow_low_precision(reason="D feeds lp matmul"):
                        nc.scalar.activation(out=d_sb[:, eod, :], in_=psd,
                                             func=AF.Identity,
                                             bias=a2r_sb[:, eod:eod + 1],
                                             scale=(1.0 / 32 if "d" in fp8
                                                    else 1.0))

                # scores^T = X @ D -> exp; Z on PE (fp8 pairs, 2-chunk lag)
                # or DVE running sum (bf16).
                for ko in range(KO if "s" in stages else 0):
                    pss = ps_sc.tile([P, QB], F32, tag="pss", bufs=2, name="pss")
                    if "s" in fp8:
                        for ec in range(4):
                            nc.tensor.matmul(
                                pss,
                                lhsT=sx[:, 2 * ec:2 * ec + 2,
                                        ko * P:(ko + 1) * P],
                                rhs=d_sb[:, 2 * ec:2 * ec + 2, :],
                                start=(ec == 0), stop=(ec == 3), perf_mode=DR)
                    else:
                        for eo in range(EO):
                            nc.tensor.matmul(
                                pss, lhsT=sx[:, eo, ko * P:(ko + 1) * P],
                                rhs=d_sb[:, eo, :],
                                start=(eo == 0), stop=(eo == EO - 1))
                    with nc.allow_low_precision(
                            reason="softmax weights feed lp AV matmul"):
                        nc.scalar.activation(out=exp_sb[:, ko, :], in_=pss,
                                             func=AF.Exp, scale=float(SCALE))
                    if Z_BF16 and "a" in fp8:
                        with nc.allow_low_precision(reason="dbg z chain"):
                            nc.scalar.activation(out=expz_sb[:, ko, :], in_=pss,
                                                 func=AF.Exp, scale=float(SCALE))
                            if ko == 0:
                                nc.vector.tensor_copy(out=acc,
                                                      in_=expz_sb[:, 0, :])
                            else:
                                nc.vector.tensor_add(out=acc, in0=acc,
                                                     in1=expz_sb[:, ko, :])
                    elif "a" in fp8:
                        # Z pair-matmul for exp pair (2c, 2c+1), emitted with
                        # a 2-chunk lag so the PE never waits on the exp act.
                        if ko >= 3 and ko % 2 == 1:
                            c = (ko - 3) // 2
                            nc.tensor.matmul(
                                psz, lhsT=ones8_sb,
                                rhs=exp_sb[:, 2 * c:2 * c + 2, :],
                                start=(c == 0), stop=False, perf_mode=DR)
                    else:
                        with nc.allow_low_precision(reason="Z acc fp32 bits"):
                            if ko == 0:
                                nc.vector.tensor_copy(out=acc,
                                                      in_=exp_sb[:, 0, :])
                            else:
                                nc.vector.tensor_add(out=acc, in0=acc,
                                                     in1=exp_sb[:, ko, :])

                # AV + out-proj; Z tail + zb broadcast emitted inside the
                # AV stream so the PE reaches them after their inputs land.
                zinv = (None if ("a" in fp8 and not Z_BF16) else
                        blk.tile([1, QB], F32R, tag="zinv", bufs=1,
                                 name="zinv"))
                zb_sb = blk.tile([P, QB], F32, tag="zb", bufs=1, name="zb_sb")
                for ft in range(2 if "a" in stages else 0):
                    psp = [ps_mid.tile([P, QB], F32, tag="psa", bufs=6,
                                       name=f"psa{j}") for j in range(4)]
                    for kc in range(8 if "a" in fp8 else KO):
                        for j in range(4):
                            if "a" in fp8:
                                nc.tensor.matmul(
                                    psp[j],
                                    lhsT=u_sb[:, 2 * kc:2 * kc + 2,
                                              ft * 512 + j * P:
                                              ft * 512 + (j + 1) * P],
                                    rhs=exp_sb[:, 2 * kc:2 * kc + 2, :],
                                    start=(kc == 0), stop=(kc == 7),
                                    perf_mode=DR)
                            else:
                                nc.tensor.matmul(
                                    psp[j],
                                    lhsT=u_sb[:, kc, ft * 512 + j * P:
                                              ft * 512 + (j + 1) * P],
                                    rhs=exp_sb[:, kc, :],
                                    start=(kc == 0), stop=(kc == KO - 1))
                        if ft == 0 and kc == 1:
                            if "a" in fp8 and not Z_BF16:
                                # Z tail pair (c=7; c=0..6 were interleaved
                                # into the scores stream); psz rows are all Z
                                # (all-ones weights), so one reciprocal yields
                                # the broadcast 1/Z.
                                for c in (7,):
                                    nc.tensor.matmul(
                                        psz, lhsT=ones8_sb,
                                        rhs=exp_sb[:, 2 * c:2 * c + 2, :],
                                        start=False, stop=(c == 7),
                                        perf_mode=DR)
                                nc.vector.reciprocal(out=zb_sb, in_=psz)
                            else:
                                nc.tensor.matmul(psz[:1, :],
                                                 lhsT=ones_sb[:, 0:1],
                                                 rhs=acc, start=True, stop=True)
                                with nc.allow_low_precision(
                                        reason="zinv feeds f32r matmul"):
                                    nc.vector.reciprocal(out=zinv[:1, :],
                                                         in_=psz[:1, :])
                        if ft == 0 and kc == 3 and ("a" not in fp8
                                                    or Z_BF16):
                            psb = ps_sc.tile([P, QB], F32, tag="pss", bufs=2,
                                             name="psb")
                            nc.tensor.matmul(psb, lhsT=ones_sb[:1, :],
                                             rhs=zinv[:1, :],
                                             start=True, stop=True)
                            nc.vector.tensor_copy(out=zb_sb, in_=psb)
                    for j in range(4):
                        fo = ft * 4 + j
                        osa = blk.tile([P, QB], F32, tag="osa", bufs=2,
                                       name="osa")
                        nc.vector.tensor_mul(out=osa, in0=psp[j], in1=zb_sb)
                        ost = blk.tile([P, QB], F32, tag="ost", bufs=2,
                                       name="ost")
                        nc.scalar.activation(out=ost, in_=osa, func=AF.Identity,
                                             bias=bor_sb[:, fo:fo + 1],
                                             scale=1.0)
                        # stores ride the Activation HWDGE queue
                        nc.scalar.dma_start(
                            out=out_ap[fo * P:(fo + 1) * P, q0:q0 + QB],
                            in_=ost)

            blk.release()
            ps_sc.release()
            ps_mid.release()

        if loop_iters is None:
            body()
        else:
            with tc.For_i(0, loop_iters):
                body()

        persist.release()

    nc.compile()
    return nc


def _prep_shared(Wq, bq, Wk, bk, Wv, bv, Wo, bo, fp8=FP8_STAGES):
    def chunk_w(W, free, dt):
        wT = np.ascontiguousarray(np.asarray(W, dtype=np.float32).T)
        n = E // free
        return np.ascontiguousarray(
            wT.reshape(EO, P, n, free).transpose(2, 1, 0, 3).astype(dt))

    W64 = {k: np.asarray(v, dtype=np.float64)
           for k, v in dict(Wq=Wq, bq=bq, Wk=Wk, Wv=Wv, bv=bv, Wo=Wo,
                            bo=bo).items()}
    # Q.K^T and V.Wo^T weight fusions (see module docstring)
    M = (W64["Wk"].T @ W64["Wq"]).astype(np.float32)      # [e, e']
    G = (W64["Wo"] @ W64["Wv"]).astype(np.float32)        # [f, e']
    a2 = (W64["Wk"].T @ W64["bq"]).astype(np.float32)     # [e]
    bo_folded = (W64["bo"] + W64["Wo"] @ W64["bv"]).astype(np.float32)
    return {
        "mt": chunk_w(M * 32 if "d" in fp8 else M, P,
                      F8NP if "d" in fp8 else BFNP),
        "wv": chunk_w(G * 32 if "v" in fp8 else G, 512,
                      F8NP if "v" in fp8 else BFNP),
        "a2r": np.ascontiguousarray(a2.reshape(EO, P).T),
        "bor": np.ascontiguousarray(bo_folded.reshape(FO, P).T),
        "ones": np.ones((P, P), dtype=np.float32),
        "ones8": np.ones((P, 2, P), dtype=F8NP),
    }


def make_in_maps(x, Wq, bq, Wk, bk, Wv, bv, Wo, bo, fp8=FP8_STAGES):
    shared = _prep_shared(Wq, bq, Wk, bk, Wv, bv, Wo, bo, fp8)
    in_maps = []
    for c in range(N_CORES):
        b, h = c // 2, c % 2
        xt = np.asarray(x[b]).T  # [E, S]
        if h == 0:
            xt_p = np.ascontiguousarray(xt)
        else:
            xt_p = np.ascontiguousarray(
                np.concatenate([xt[:, SH:], xt[:, :SH]], axis=1))
        m = {"xt": xt_p.astype(BFNP), "xt8": xt_p.astype(F8NP)}
        m.update(shared)
        in_maps.append(m)
    return in_maps


def kernel(x, Wq, bq, Wk, bk, Wv, bv, Wo, bo):
    x = np.asarray(x, dtype=np.float32)
    args = [np.asarray(a, dtype=np.float32)
            for a in (Wq, bq, Wk, bk, Wv, bv, Wo, bo)]
    if "nc" not in _CACHE:
        _CACHE["nc"] = build_nc()
    nc = _CACHE["nc"]
    in_maps = make_in_maps(x, *args)
    res = bass_utils.run_bass_kernel_spmd(nc, in_maps,
                                          core_ids=list(range(N_CORES)))
    out = np.empty((B, S, E), dtype=np.float32)
    for c in range(N_CORES):
        b, h = c // 2, c % 2
        out[b, h * SH:(h + 1) * SH, :] = res.results[c]["out"].T
    return out


# revision 17
# speedup vs baseline: 6.0969x; 1.0094x over previous
"""Multi-head self-attention (full-embed, no head split) on 8 Trainium2 cores.

Sharding: data-parallel over (batch=4) x (query-half=2) = 8 cores.
Each core computes U for the full 2048-row sequence of its batch (duplicated
across the core pair), attention scores for its 1024 query rows, softmax,
weights @ U, and the (folded) output projection for its rows.

Weight folds (host-side, fp64):
  - Q/K projections fused into M = Wk^T @ Wq; on device D = M . x^T, then
    scores^T = X @ D.  Score bias: q-dependent parts cancel in softmax; the
    k-part folds into D's per-partition bias (a2 = Wk^T @ bq).
  - V and out_proj fused into G = Wo @ Wv; U = X @ G^T so the AV matmul
    directly yields the final output.  V bias folds into the output bias
    (bo' = bo + Wo @ bv, exact since softmax weights sum to 1).

Precision/engine choices:
  - Matmul stages can run in fp8e4m3 with MatmulPerfMode.DoubleRow: operand
    APs are [128, 2, free] pairs of adjacent contraction chunks, which the
    existing [p, chunk, free] tile layouts provide by plain slicing.  This
    halves both the PE row count and the matmul instruction count (measured
    ~82ns fixed overhead per matmul instruction on HW).  Stage set is
    FP8_STAGES ("v" V-proj, "d" D, "s" scores, "a" AV); non-fp8 stages use
    bf16 (fp32 PSUM accumulation everywhere).
  - Everything stays resident in SBUF (no DRAM spill): xt, mt, wv, U, exp.
    Loads ride the SP HWDGE queue in consumption order; output stores ride
    the Activation queue so they never head-of-line-block the next
    iteration's prefetch (relevant for the For_i timing loop / back-to-back
    calls).
  - Softmax denominator: when AV is fp8, Z accumulates on the PE as
    ones-pair DoubleRow matmuls interleaved into the scores stream (2-chunk
    lag, no stall); otherwise a DVE running sum.  The reciprocal broadcast
    (zb) matmuls are emitted inside the AV stream so the PE never waits.
  - Output is computed transposed (out^T [f, q], per-partition bias) and
    transposed back on host.
"""
import sys

sys.path.insert(0, '/opt/trn_rl_repo')

import numpy as np
import ml_dtypes

import concourse.bass as bass
import concourse.bacc as bacc
import concourse.tile as tile
import concourse.mybir as mybir
from concourse import bass_utils

F32 = mybir.dt.float32
F32R = mybir.dt.float32r
BF16 = mybir.dt.bfloat16
FP8 = mybir.dt.float8e4
AF = mybir.ActivationFunctionType
DR = mybir.MatmulPerfMode.DoubleRow
BFNP = ml_dtypes.bfloat16
F8NP = ml_dtypes.float8_e4m3

N_CORES = 8
B, S, E = 4, 2048, 1024
SH = S // 2          # per-core query rows
P = 128
EO = E // P          # 8 contraction chunks
FO = E // P          # 8 output-feature chunks
KO = S // P          # 16 key chunks
QB = 512             # q block (PSUM free dim)
NQB = SH // QB       # 2 q blocks per core
SCALE = 1.0 / np.sqrt(np.float32(E))

# Which matmul stages run fp8e4m3 + DoubleRow.  Speed/accuracy measured on
# HW (gate: rel_err < 2e-2; errors are run-to-run deterministic):
#   ""     254.9us  1.79e-3   (all-bf16)
#   "a"    228.5us  1.83e-2   <- shipped: AV + Z in fp8
#   "v"    254.7us  1.69e-2   (no speed gain)
#   "s"    252.0us  1.31e-2   (no speed gain)
#   "vdsa" 152.1us  3.09e-2   (fails gate; fp8 stage errors RSS-combine)
FP8_STAGES = "a"
Z_BF16 = False       # debug: force bf16 DVE-chain Z even with fp8 AV

_CACHE = {}


def build_nc(loop_iters=None, stages="vdsa", fp8=FP8_STAGES):
    """Build + compile the Bass module. loop_iters wraps the whole body in a
    hardware loop (used only for timing amplification by test harnesses).
    stages: subset of "vdsa" for timing bisection.  fp8: subset of "vdsa"
    running fp8+DoubleRow."""
    nc = bacc.Bacc("TRN2", target_bir_lowering=False, debug=False,
                   enable_asserts=False, num_devices=N_CORES)

    need_xt16 = not ({"v", "d", "s"} <= set(fp8))
    need_xt8 = bool({"v", "d", "s"} & set(fp8))

    aps = {}
    if need_xt16:
        aps["xt"] = nc.dram_tensor("xt", [E, S], BF16, kind="ExternalInput").ap()
    if need_xt8:
        aps["xt8"] = nc.dram_tensor("xt8", [E, S], FP8, kind="ExternalInput").ap()
    aps["mt"] = nc.dram_tensor("mt", [EO, P, EO, P], FP8 if "d" in fp8 else BF16,
                               kind="ExternalInput").ap()
    aps["wv"] = nc.dram_tensor("wv", [2, P, EO, 512], FP8 if "v" in fp8 else BF16,
                               kind="ExternalInput").ap()
    a2r_ap = nc.dram_tensor("a2r", [P, EO], F32, kind="ExternalInput").ap()
    bor_ap = nc.dram_tensor("bor", [P, FO], F32, kind="ExternalInput").ap()
    ones_ap = nc.dram_tensor("ones", [P, P], F32R, kind="ExternalInput").ap()
    if "a" in fp8:
        ones8_ap = nc.dram_tensor("ones8", [P, 2, P], FP8, kind="ExternalInput").ap()
    # transposed output; host transposes back
    out_ap = nc.dram_tensor("out", [E, SH], F32, kind="ExternalOutput").ap()

    d_dt = FP8 if "s" in fp8 else BF16
    e_dt = FP8 if "a" in fp8 else BF16

    with tile.TileContext(nc) as tc:
        persist = tc.alloc_tile_pool(name="persist", bufs=1)

        def body():
            xt_sb = (persist.tile([P, EO, S], BF16, name="xt_sb")
                     if need_xt16 else None)
            xt8_sb = (persist.tile([P, EO, S], FP8, name="xt8_sb")
                      if need_xt8 else None)
            mt_sb = persist.tile([P, EO, EO, P], FP8 if "d" in fp8 else BF16,
                                 name="mt_sb")
            wv_sb = persist.tile([P, 2, EO, 512], FP8 if "v" in fp8 else BF16,
                                 name="wv_sb")
            u_sb = persist.tile([P, KO, E], e_dt, name="u_sb")
            ones_sb = persist.tile([P, P], F32R, name="ones_sb")
            ones8_sb = (persist.tile([P, 2, P], FP8, name="ones8_sb")
                        if "a" in fp8 else None)
            a2r_sb = persist.tile([P, EO], F32, name="a2r_sb")
            bor_sb = persist.tile([P, FO], F32, name="bor_sb")

            vx = xt8_sb if "v" in fp8 else xt_sb
            dx = xt8_sb if "d" in fp8 else xt_sb
            sx = xt8_sb if "s" in fp8 else xt_sb

            # ---- loads (SP queue) in phase-1 consumption order.
            def load_xt(dst, src):
                for eo in range(EO):
                    nc.sync.dma_start(out=dst[:, eo, 0:512],
                                      in_=src[eo * P:(eo + 1) * P, 0:512])
                for q4 in range(1, 4):
                    for eo in range(EO):
                        nc.sync.dma_start(
                            out=dst[:, eo, q4 * 512:(q4 + 1) * 512],
                            in_=src[eo * P:(eo + 1) * P,
                                    q4 * 512:(q4 + 1) * 512])

            for eo in range(EO):
                nc.sync.dma_start(out=wv_sb[:, 0, eo, :], in_=aps["wv"][0, :, eo, :])
            if need_xt8:
                load_xt(xt8_sb, aps["xt8"])
            if need_xt16:
                load_xt(xt_sb, aps["xt"])
            nc.sync.dma_start(out=wv_sb[:, 1], in_=aps["wv"][1])
            for eod in range(EO):
                nc.sync.dma_start(out=mt_sb[:, eod], in_=aps["mt"][eod])
            nc.sync.dma_start(out=a2r_sb, in_=a2r_ap)
            nc.sync.dma_start(out=bor_sb, in_=bor_ap)
            nc.sync.dma_start(out=ones_sb, in_=ones_ap)
            if "a" in fp8:
                nc.sync.dma_start(out=ones8_sb, in_=ones8_ap)

            # ---- Phase 1: U = X @ G^T (natural [s, f]), fully resident.
            psB = tc.alloc_tile_pool(name="psB", bufs=1, space="PSUM")
            for ft in range(2 if "v" in stages else 0):
                for grp in range(4):
                    psv = [psB.tile([P, 512], F32, tag="psv", bufs=8,
                                    name=f"psv{j}") for j in range(4)]
                    if "v" in fp8:
                        for ec in range(4):
                            for j in range(4):
                                so = grp * 4 + j
                                nc.tensor.matmul(
                                    psv[j],
                                    lhsT=vx[:, 2 * ec:2 * ec + 2,
                                            so * P:(so + 1) * P],
                                    rhs=wv_sb[:, ft, 2 * ec:2 * ec + 2, :],
                                    start=(ec == 0), stop=(ec == 3),
                                    perf_mode=DR)
                    else:
                        for eo in range(EO):
                            for j in range(4):
                                so = grp * 4 + j
                                nc.tensor.matmul(
                                    psv[j],
                                    lhsT=vx[:, eo, so * P:(so + 1) * P],
                                    rhs=wv_sb[:, ft, eo, :],
                                    start=(eo == 0), stop=(eo == EO - 1))
                    usc = 1.0 / 32 if "v" in fp8 else 1.0
                    for j in range(4):
                        so = grp * 4 + j
                        dst = u_sb[:, so, ft * 512:(ft + 1) * 512]
                        with nc.allow_low_precision(
                                reason="U feeds low-precision AV matmul"):
                            if "a" in fp8 or j % 2 == 0:
                                # fp8 stores must go via the Activation
                                # engine (DVE fp8 writes corrupt)
                                nc.scalar.activation(out=dst, in_=psv[j],
                                                     func=AF.Identity, scale=usc)
                            elif "v" in fp8:
                                nc.vector.tensor_scalar_mul(out=dst,
                                                            in0=psv[j],
                                                            scalar1=usc)
                            else:
                                nc.vector.tensor_copy(out=dst, in_=psv[j])
            psB.release()

            # ---- Phase 2: per q-block attention + folded output projection.
            ps_mid = tc.alloc_tile_pool(name="ps_mid", bufs=1, space="PSUM")
            ps_sc = tc.alloc_tile_pool(name="ps_sc", bufs=1, space="PSUM")
            blk = tc.alloc_tile_pool(name="blk", bufs=1)

            for qb in range(NQB):
                q0 = qb * QB
                exp_sb = blk.tile([P, KO, QB], e_dt, tag="exp", bufs=1,
                                  name="exp_sb")
                expz_sb = (blk.tile([P, KO, QB], BF16, tag="expz", bufs=1,
                                    name="expz_sb")
                           if (Z_BF16 and "a" in fp8) else None)
                d_sb = blk.tile([P, EO, QB], d_dt, tag="d", bufs=1, name="d_sb")
                acc = (None if ("a" in fp8 and not Z_BF16) else
                       blk.tile([P, QB], F32R, tag="acc", bufs=1, name="acc"))
                psz = ps_mid.tile([P, QB], F32, tag="psa", bufs=6, name="psz")

                # D = M . x^T (+ a2 per-partition bias)
                for eod in range(EO if "d" in stages else 0):
                    psd = ps_sc.tile([P, QB], F32, tag="pss", bufs=2, name="psd")
                    if "d" in fp8:
                        for ec in range(4):
                            nc.tensor.matmul(
                                psd, lhsT=mt_sb[:, eod, 2 * ec:2 * ec + 2, :],
                                rhs=dx[:, 2 * ec:2 * ec + 2, q0:q0 + QB],
                                start=(ec == 0), stop=(ec == 3), perf_mode=DR)
                    else:
                        for eo in range(EO):
                            nc.tensor.matmul(psd, lhsT=mt_sb[:, eod, eo, :],
                                             rhs=dx[:, eo, q0:q0 + QB],
                                             start=(eo == 0), stop=(eo == EO - 1))
                    with nc.allow_low_precision(reason="D feeds lp matmul"):
                        nc.scalar.activation(out=d_sb[:, eod, :], in_=psd,
                                             func=AF.Identity,
                                             bias=a2r_sb[:, eod:eod + 1],
                                             scale=(1.0 / 32 if "d" in fp8
                                                    else 1.0))

                # scores^T = X @ D -> exp; Z on PE (fp8 pairs, 2-chunk lag)
                # or DVE running sum (bf16).
                for ko in range(KO if "s" in stages else 0):
                    pss = ps_sc.tile([P, QB], F32, tag="pss", bufs=2, name="pss")
                    if "s" in fp8:
                        for ec in range(4):
                            nc.tensor.matmul(
                                pss,
                                lhsT=sx[:, 2 * ec:2 * ec + 2,
                                        ko * P:(ko + 1) * P],
                                rhs=d_sb[:, 2 * ec:2 * ec + 2, :],
                                start=(ec == 0), stop=(ec == 3), perf_mode=DR)
                    else:
                        for eo in range(EO):
                            nc.tensor.matmul(
                                pss, lhsT=sx[:, eo, ko * P:(ko + 1) * P],
                                rhs=d_sb[:, eo, :],
                                start=(eo == 0), stop=(eo == EO - 1))
                    with nc.allow_low_precision(
                            reason="softmax weights feed lp AV matmul"):
                        nc.scalar.activation(out=exp_sb[:, ko, :], in_=pss,
                                             func=AF.Exp, scale=float(SCALE))
                    if Z_BF16 and "a" in fp8:
                        with nc.allow_low_precision(reason="dbg z chain"):
                            nc.scalar.activation(out=expz_sb[:, ko, :], in_=pss,
                                                 func=AF.Exp, scale=float(SCALE))
                            if ko == 0:
                                nc.vector.tensor_copy(out=acc,
                                                      in_=expz_sb[:, 0, :])
                            else:
                                nc.vector.tensor_add(out=acc, in0=acc,
                                                     in1=expz_sb[:, ko, :])
                    elif "a" in fp8:
                        # Z pair-matmul for exp pair (2c, 2c+1), emitted with
                        # a 2-chunk lag so the PE never waits on the exp act.
                        if ko >= 3 and ko % 2 == 1:
                            c = (ko - 3) // 2
                            nc.tensor.matmul(
                                psz, lhsT=ones8_sb,
                                rhs=exp_sb[:, 2 * c:2 * c + 2, :],
                                start=(c == 0), stop=False, perf_mode=DR)
                    else:
                        with nc.allow_low_precision(reason="Z acc fp32 bits"):
                            if ko == 0:
                                nc.vector.tensor_copy(out=acc,
                                                      in_=exp_sb[:, 0, :])
                            else:
                                nc.vector.tensor_add(out=acc, in0=acc,
                                                     in1=exp_sb[:, ko, :])

                # AV + out-proj; Z tail + zb broadcast emitted inside the
                # AV stream so the PE reaches them after their inputs land.
                zinv = (None if ("a" in fp8 and not Z_BF16) else
                        blk.tile([1, QB], F32R, tag="zinv", bufs=1,
                                 name="zinv"))
                zb_sb = blk.tile([P, QB], F32, tag="zb", bufs=1, name="zb_sb")
                def emit_z_tail():
                    if "a" in fp8 and not Z_BF16:
                        # Z tail pair (c=7; c=0..6 were interleaved into the
                        # scores stream); psz rows are all Z (all-ones
                        # weights), so one reciprocal yields the broadcast
                        # 1/Z.
                        nc.tensor.matmul(psz, lhsT=ones8_sb,
                                         rhs=exp_sb[:, 14:16, :],
                                         start=False, stop=True, perf_mode=DR)
                        nc.vector.reciprocal(out=zb_sb, in_=psz)
                    else:
                        nc.tensor.matmul(psz[:1, :], lhsT=ones_sb[:, 0:1],
                                         rhs=acc, start=True, stop=True)
                        with nc.allow_low_precision(
                                reason="zinv feeds f32r matmul"):
                            nc.vector.reciprocal(out=zinv[:1, :],
                                                 in_=psz[:1, :])

                def emit_zb():
                    if "a" in fp8 and not Z_BF16:
                        return
                    psb = ps_sc.tile([P, QB], F32, tag="pss", bufs=2,
                                     name="psb")
                    nc.tensor.matmul(psb, lhsT=ones_sb[:1, :],
                                     rhs=zinv[:1, :], start=True, stop=True)
                    nc.vector.tensor_copy(out=zb_sb, in_=psb)

                # K-dense AV: one PSUM bank accumulates its full contraction
                # before the next starts (measured ~30ns/mm cheaper than
                # bank-interleaved issue).  The Z tail rides after the first
                # bank so the PE never waits on the last exp activation.
                for ft in range(2 if "a" in stages else 0):
                    psp = [ps_mid.tile([P, QB], F32, tag="psa", bufs=6,
                                       name=f"psa{j}") for j in range(4)]
                    for j in range(4):
                        for kc in range(8 if "a" in fp8 else KO):
                            if "a" in fp8:
                                nc.tensor.matmul(
                                    psp[j],
                                    lhsT=u_sb[:, 2 * kc:2 * kc + 2,
                                              ft * 512 + j * P:
                                              ft * 512 + (j + 1) * P],
                                    rhs=exp_sb[:, 2 * kc:2 * kc + 2, :],
                                    start=(kc == 0), stop=(kc == 7),
                                    perf_mode=DR)
                            else:
                                nc.tensor.matmul(
                                    psp[j],
                                    lhsT=u_sb[:, kc, ft * 512 + j * P:
                                              ft * 512 + (j + 1) * P],
                                    rhs=exp_sb[:, kc, :],
                                    start=(kc == 0), stop=(kc == KO - 1))
                        if ft == 0 and j == 0:
                            emit_z_tail()
                        if ft == 0 and j == 1:
                            emit_zb()
                    for j in range(4):
                        fo = ft * 4 + j
                        osa = blk.tile([P, QB], F32, tag="osa", bufs=2,
                                       name="osa")
                        nc.vector.tensor_mul(out=osa, in0=psp[j], in1=zb_sb)
                        ost = blk.tile([P, QB], F32, tag="ost", bufs=2,
                                       name="ost")
                        nc.scalar.activation(out=ost, in_=osa, func=AF.Identity,
                                             bias=bor_sb[:, fo:fo + 1],
                                             scale=1.0)
                        # stores ride the Activation HWDGE queue
                        nc.scalar.dma_start(
                            out=out_ap[fo * P:(fo + 1) * P, q0:q0 + QB],
                            in_=ost)

            blk.release()
            ps_sc.release()
            ps_mid.release()

        if loop_iters is None:
            body()
        else:
            with tc.For_i(0, loop_iters):
                body()

        persist.release()

    nc.compile()
    return nc


def _prep_shared(Wq, bq, Wk, bk, Wv, bv, Wo, bo, fp8=FP8_STAGES):
    def chunk_w(W, free, dt):
        wT = np.ascontiguousarray(np.asarray(W, dtype=np.float32).T)
        n = E // free
        return np.ascontiguousarray(
            wT.reshape(EO, P, n, free).transpose(2, 1, 0, 3).astype(dt))

    W64 = {k: np.asarray(v, dtype=np.float64)
           for k, v in dict(Wq=Wq, bq=bq, Wk=Wk, Wv=Wv, bv=bv, Wo=Wo,
                            bo=bo).items()}
    # Q.K^T and V.Wo^T weight fusions (see module docstring)
    M = (W64["Wk"].T @ W64["Wq"]).astype(np.float32)      # [e, e']
    G = (W64["Wo"] @ W64["Wv"]).astype(np.float32)        # [f, e']
    a2 = (W64["Wk"].T @ W64["bq"]).astype(np.float32)     # [e]
    bo_folded = (W64["bo"] + W64["Wo"] @ W64["bv"]).astype(np.float32)
    return {
        "mt": chunk_w(M * 32 if "d" in fp8 else M, P,
                      F8NP if "d" in fp8 else BFNP),
        "wv": chunk_w(G * 32 if "v" in fp8 else G, 512,
                      F8NP if "v" in fp8 else BFNP),
        "a2r": np.ascontiguousarray(a2.reshape(EO, P).T),
        "bor": np.ascontiguousarray(bo_folded.reshape(FO, P).T),
        "ones": np.ones((P, P), dtype=np.float32),
        "ones8": np.ones((P, 2, P), dtype=F8NP),
    }


def make_in_maps(x, Wq, bq, Wk, bk, Wv, bv, Wo, bo, fp8=FP8_STAGES):
    shared = _prep_shared(Wq, bq, Wk, bk, Wv, bv, Wo, bo, fp8)
    in_maps = []
    for c in range(N_CORES):
        b, h = c // 2, c % 2
        xt = np.asarray(x[b]).T  # [E, S]
        if h == 0:
            xt_p = np.ascontiguousarray(xt)
        else:
            xt_p = np.ascontiguousarray(
                np.concatenate([xt[:, SH:], xt[:, :SH]], axis=1))
        m = {"xt": xt_p.astype(BFNP), "xt8": xt_p.astype(F8NP)}
        m.update(shared)
        in_maps.append(m)
    return in_maps


def kernel(x, Wq, bq, Wk, bk, Wv, bv, Wo, bo):
    x = np.asarray(x, dtype=np.float32)
    args = [np.asarray(a, dtype=np.float32)
            for a in (Wq, bq, Wk, bk, Wv, bv, Wo, bo)]
    if "nc" not in _CACHE:
        _CACHE["nc"] = build_nc()
    nc = _CACHE["nc"]
    in_maps = make_in_maps(x, *args)
    res = bass_utils.run_bass_kernel_spmd(nc, in_maps,
                                          core_ids=list(range(N_CORES)))
    out = np.empty((B, S, E), dtype=np.float32)
    for c in range(N_CORES):
        b, h = c // 2, c % 2
        out[b, h * SH:(h + 1) * SH, :] = res.results[c]["out"].T
    return out


# revision 18
# speedup vs baseline: 6.0974x; 1.0001x over previous
"""Multi-head self-attention (full-embed, no head split) on 8 Trainium2 cores.

Sharding: data-parallel over (batch=4) x (query-half=2) = 8 cores.
Each core computes U for the full 2048-row sequence of its batch (duplicated
across the core pair), attention scores for its 1024 query rows, softmax,
weights @ U, and the (folded) output projection for its rows.

Weight folds (host-side, fp64):
  - Q/K projections fused into M = Wk^T @ Wq; on device D = M . x^T, then
    scores^T = X @ D.  Score bias: q-dependent parts cancel in softmax; the
    k-part folds into D's per-partition bias (a2 = Wk^T @ bq).
  - V and out_proj fused into G = Wo @ Wv; U = X @ G^T so the AV matmul
    directly yields the final output.  V bias folds into the output bias
    (bo' = bo + Wo @ bv, exact since softmax weights sum to 1).

Precision/engine choices:
  - Matmul stages can run in fp8e4m3 with MatmulPerfMode.DoubleRow: operand
    APs are [128, 2, free] pairs of adjacent contraction chunks, which the
    existing [p, chunk, free] tile layouts provide by plain slicing.  This
    halves both the PE row count and the matmul instruction count (measured
    ~82ns fixed overhead per matmul instruction on HW).  Stage set is
    FP8_STAGES ("v" V-proj, "d" D, "s" scores, "a" AV); non-fp8 stages use
    bf16 (fp32 PSUM accumulation everywhere).
  - Everything stays resident in SBUF (no DRAM spill): xt, mt, wv, U, exp.
    Loads ride the SP HWDGE queue in consumption order; output stores ride
    the Activation queue so they never head-of-line-block the next
    iteration's prefetch (relevant for the For_i timing loop / back-to-back
    calls).
  - Softmax denominator: when AV is fp8, Z accumulates on the PE as
    ones-pair DoubleRow matmuls interleaved into the scores stream (2-chunk
    lag, no stall); otherwise a DVE running sum.  The reciprocal broadcast
    (zb) matmuls are emitted inside the AV stream so the PE never waits.
  - Output is computed transposed (out^T [f, q], per-partition bias) and
    transposed back on host.
"""
import sys

sys.path.insert(0, '/opt/trn_rl_repo')

import numpy as np
import ml_dtypes

import concourse.bass as bass
import concourse.bacc as bacc
import concourse.tile as tile
import concourse.mybir as mybir
from concourse import bass_utils

F32 = mybir.dt.float32
F32R = mybir.dt.float32r
BF16 = mybir.dt.bfloat16
FP8 = mybir.dt.float8e4
AF = mybir.ActivationFunctionType
DR = mybir.MatmulPerfMode.DoubleRow
BFNP = ml_dtypes.bfloat16
F8NP = ml_dtypes.float8_e4m3

N_CORES = 8
B, S, E = 4, 2048, 1024
SH = S // 2          # per-core query rows
P = 128
EO = E // P          # 8 contraction chunks
FO = E // P          # 8 output-feature chunks
KO = S // P          # 16 key chunks
QB = 512             # q block (PSUM free dim)
NQB = SH // QB       # 2 q blocks per core
SCALE = 1.0 / np.sqrt(np.float32(E))

# Which matmul stages run fp8e4m3 + DoubleRow.  Speed/accuracy measured on
# HW (gate: rel_err < 2e-2; errors are run-to-run deterministic):
#   ""     254.9us  1.79e-3   (all-bf16)
#   "a"    228.5us  1.83e-2   <- shipped: AV + Z in fp8
#   "v"    254.7us  1.69e-2   (no speed gain)
#   "s"    252.0us  1.31e-2   (no speed gain)
#   "vdsa" 152.1us  3.09e-2   (fails gate; fp8 stage errors RSS-combine)
FP8_STAGES = "a"
# Z via a bf16 DVE running sum (exp double-written bf16+fp8) instead of PE
# ones-pair matmuls: trades 16 PE matmuls (each with a 256-col LDWEIGHTS)
# for ACT/DVE work that hides under the PE stream.  Measured 223.7us /
# 1.8342e-2 vs 225.8us / 1.8327e-2 for the PE-side Z.
Z_BF16 = True

_CACHE = {}


def build_nc(loop_iters=None, stages="vdsa", fp8=FP8_STAGES):
    """Build + compile the Bass module. loop_iters wraps the whole body in a
    hardware loop (used only for timing amplification by test harnesses).
    stages: subset of "vdsa" for timing bisection.  fp8: subset of "vdsa"
    running fp8+DoubleRow."""
    nc = bacc.Bacc("TRN2", target_bir_lowering=False, debug=False,
                   enable_asserts=False, num_devices=N_CORES)

    need_xt16 = not ({"v", "d", "s"} <= set(fp8))
    need_xt8 = bool({"v", "d", "s"} & set(fp8))

    aps = {}
    if need_xt16:
        aps["xt"] = nc.dram_tensor("xt", [E, S], BF16, kind="ExternalInput").ap()
    if need_xt8:
        aps["xt8"] = nc.dram_tensor("xt8", [E, S], FP8, kind="ExternalInput").ap()
    aps["mt"] = nc.dram_tensor("mt", [EO, P, EO, P], FP8 if "d" in fp8 else BF16,
                               kind="ExternalInput").ap()
    aps["wv"] = nc.dram_tensor("wv", [2, P, EO, 512], FP8 if "v" in fp8 else BF16,
                               kind="ExternalInput").ap()
    a2r_ap = nc.dram_tensor("a2r", [P, EO], F32, kind="ExternalInput").ap()
    bor_ap = nc.dram_tensor("bor", [P, FO], F32, kind="ExternalInput").ap()
    ones_ap = nc.dram_tensor("ones", [P, P], F32R, kind="ExternalInput").ap()
    if "a" in fp8:
        ones8_ap = nc.dram_tensor("ones8", [P, 2, P], FP8, kind="ExternalInput").ap()
    # transposed output; host transposes back
    out_ap = nc.dram_tensor("out", [E, SH], F32, kind="ExternalOutput").ap()

    d_dt = FP8 if "s" in fp8 else BF16
    e_dt = FP8 if "a" in fp8 else BF16

    with tile.TileContext(nc) as tc:
        persist = tc.alloc_tile_pool(name="persist", bufs=1)

        def body():
            xt_sb = (persist.tile([P, EO, S], BF16, name="xt_sb")
                     if need_xt16 else None)
            xt8_sb = (persist.tile([P, EO, S], FP8, name="xt8_sb")
                      if need_xt8 else None)
            mt_sb = persist.tile([P, EO, EO, P], FP8 if "d" in fp8 else BF16,
                                 name="mt_sb")
            wv_sb = persist.tile([P, 2, EO, 512], FP8 if "v" in fp8 else BF16,
                                 name="wv_sb")
            u_sb = persist.tile([P, KO, E], e_dt, name="u_sb")
            ones_sb = persist.tile([P, P], F32R, name="ones_sb")
            ones8_sb = (persist.tile([P, 2, P], FP8, name="ones8_sb")
                        if "a" in fp8 else None)
            a2r_sb = persist.tile([P, EO], F32, name="a2r_sb")
            bor_sb = persist.tile([P, FO], F32, name="bor_sb")

            vx = xt8_sb if "v" in fp8 else xt_sb
            dx = xt8_sb if "d" in fp8 else xt_sb
            sx = xt8_sb if "s" in fp8 else xt_sb

            # ---- loads (SP queue) in phase-1 consumption order.
            def load_xt(dst, src):
                for eo in range(EO):
                    nc.sync.dma_start(out=dst[:, eo, 0:512],
                                      in_=src[eo * P:(eo + 1) * P, 0:512])
                for q4 in range(1, 4):
                    for eo in range(EO):
                        nc.sync.dma_start(
                            out=dst[:, eo, q4 * 512:(q4 + 1) * 512],
                            in_=src[eo * P:(eo + 1) * P,
                                    q4 * 512:(q4 + 1) * 512])

            for eo in range(EO):
                nc.sync.dma_start(out=wv_sb[:, 0, eo, :], in_=aps["wv"][0, :, eo, :])
            if need_xt8:
                load_xt(xt8_sb, aps["xt8"])
            if need_xt16:
                load_xt(xt_sb, aps["xt"])
            nc.sync.dma_start(out=wv_sb[:, 1], in_=aps["wv"][1])
            for eod in range(EO):
                nc.sync.dma_start(out=mt_sb[:, eod], in_=aps["mt"][eod])
            nc.sync.dma_start(out=a2r_sb, in_=a2r_ap)
            nc.sync.dma_start(out=bor_sb, in_=bor_ap)
            nc.sync.dma_start(out=ones_sb, in_=ones_ap)
            if "a" in fp8:
                nc.sync.dma_start(out=ones8_sb, in_=ones8_ap)

            # ---- Phase 1: U = X @ G^T (natural [s, f]), fully resident.
            psB = tc.alloc_tile_pool(name="psB", bufs=1, space="PSUM")
            for ft in range(2 if "v" in stages else 0):
                for grp in range(4):
                    psv = [psB.tile([P, 512], F32, tag="psv", bufs=8,
                                    name=f"psv{j}") for j in range(4)]
                    if "v" in fp8:
                        for ec in range(4):
                            for j in range(4):
                                so = grp * 4 + j
                                nc.tensor.matmul(
                                    psv[j],
                                    lhsT=vx[:, 2 * ec:2 * ec + 2,
                                            so * P:(so + 1) * P],
                                    rhs=wv_sb[:, ft, 2 * ec:2 * ec + 2, :],
                                    start=(ec == 0), stop=(ec == 3),
                                    perf_mode=DR)
                    else:
                        for eo in range(EO):
                            for j in range(4):
                                so = grp * 4 + j
                                nc.tensor.matmul(
                                    psv[j],
                                    lhsT=vx[:, eo, so * P:(so + 1) * P],
                                    rhs=wv_sb[:, ft, eo, :],
                                    start=(eo == 0), stop=(eo == EO - 1))
                    usc = 1.0 / 32 if "v" in fp8 else 1.0
                    for j in range(4):
                        so = grp * 4 + j
                        dst = u_sb[:, so, ft * 512:(ft + 1) * 512]
                        with nc.allow_low_precision(
                                reason="U feeds low-precision AV matmul"):
                            if "a" in fp8 or j % 2 == 0:
                                # fp8 stores must go via the Activation
                                # engine (DVE fp8 writes corrupt)
                                nc.scalar.activation(out=dst, in_=psv[j],
                                                     func=AF.Identity, scale=usc)
                            elif "v" in fp8:
                                nc.vector.tensor_scalar_mul(out=dst,
                                                            in0=psv[j],
                                                            scalar1=usc)
                            else:
                                nc.vector.tensor_copy(out=dst, in_=psv[j])
            psB.release()

            # ---- Phase 2: per q-block attention + folded output projection.
            ps_mid = tc.alloc_tile_pool(name="ps_mid", bufs=1, space="PSUM")
            ps_sc = tc.alloc_tile_pool(name="ps_sc", bufs=1, space="PSUM")
            blk = tc.alloc_tile_pool(name="blk", bufs=1)

            for qb in range(NQB):
                q0 = qb * QB
                exp_sb = blk.tile([P, KO, QB], e_dt, tag="exp", bufs=1,
                                  name="exp_sb")
                expz_sb = (blk.tile([P, KO, QB], BF16, tag="expz", bufs=1,
                                    name="expz_sb")
                           if (Z_BF16 and "a" in fp8) else None)
                d_sb = blk.tile([P, EO, QB], d_dt, tag="d", bufs=1, name="d_sb")
                acc = (None if ("a" in fp8 and not Z_BF16) else
                       blk.tile([P, QB], F32R, tag="acc", bufs=1, name="acc"))
                psz = ps_mid.tile([P, QB], F32, tag="psa", bufs=6, name="psz")

                # D = M . x^T (+ a2 per-partition bias)
                for eod in range(EO if "d" in stages else 0):
                    psd = ps_sc.tile([P, QB], F32, tag="pss", bufs=2, name="psd")
                    if "d" in fp8:
                        for ec in range(4):
                            nc.tensor.matmul(
                                psd, lhsT=mt_sb[:, eod, 2 * ec:2 * ec + 2, :],
                                rhs=dx[:, 2 * ec:2 * ec + 2, q0:q0 + QB],
                                start=(ec == 0), stop=(ec == 3), perf_mode=DR)
                    else:
                        for eo in range(EO):
                            nc.tensor.matmul(psd, lhsT=mt_sb[:, eod, eo, :],
                                             rhs=dx[:, eo, q0:q0 + QB],
                                             start=(eo == 0), stop=(eo == EO - 1))
                    with nc.allow_low_precision(reason="D feeds lp matmul"):
                        nc.scalar.activation(out=d_sb[:, eod, :], in_=psd,
                                             func=AF.Identity,
                                             bias=a2r_sb[:, eod:eod + 1],
                                             scale=(1.0 / 32 if "d" in fp8
                                                    else 1.0))

                # scores^T = X @ D -> exp; Z on PE (fp8 pairs, 2-chunk lag)
                # or DVE running sum (bf16).
                for ko in range(KO if "s" in stages else 0):
                    pss = ps_sc.tile([P, QB], F32, tag="pss", bufs=2, name="pss")
                    if "s" in fp8:
                        for ec in range(4):
                            nc.tensor.matmul(
                                pss,
                                lhsT=sx[:, 2 * ec:2 * ec + 2,
                                        ko * P:(ko + 1) * P],
                                rhs=d_sb[:, 2 * ec:2 * ec + 2, :],
                                start=(ec == 0), stop=(ec == 3), perf_mode=DR)
                    else:
                        for eo in range(EO):
                            nc.tensor.matmul(
                                pss, lhsT=sx[:, eo, ko * P:(ko + 1) * P],
                                rhs=d_sb[:, eo, :],
                                start=(eo == 0), stop=(eo == EO - 1))
                    with nc.allow_low_precision(
                            reason="softmax weights feed lp AV matmul"):
                        nc.scalar.activation(out=exp_sb[:, ko, :], in_=pss,
                                             func=AF.Exp, scale=float(SCALE))
                    if Z_BF16 and "a" in fp8:
                        with nc.allow_low_precision(reason="dbg z chain"):
                            nc.scalar.activation(out=expz_sb[:, ko, :], in_=pss,
                                                 func=AF.Exp, scale=float(SCALE))
                            if ko == 0:
                                nc.vector.tensor_copy(out=acc,
                                                      in_=expz_sb[:, 0, :])
                            else:
                                nc.vector.tensor_add(out=acc, in0=acc,
                                                     in1=expz_sb[:, ko, :])
                    elif "a" in fp8:
                        # Z pair-matmul for exp pair (2c, 2c+1), emitted with
                        # a 2-chunk lag so the PE never waits on the exp act.
                        if ko >= 3 and ko % 2 == 1:
                            c = (ko - 3) // 2
                            nc.tensor.matmul(
                                psz, lhsT=ones8_sb,
                                rhs=exp_sb[:, 2 * c:2 * c + 2, :],
                                start=(c == 0), stop=False, perf_mode=DR)
                    else:
                        with nc.allow_low_precision(reason="Z acc fp32 bits"):
                            if ko == 0:
                                nc.vector.tensor_copy(out=acc,
                                                      in_=exp_sb[:, 0, :])
                            else:
                                nc.vector.tensor_add(out=acc, in0=acc,
                                                     in1=exp_sb[:, ko, :])

                # AV + out-proj; Z tail + zb broadcast emitted inside the
                # AV stream so the PE reaches them after their inputs land.
                zinv = (None if ("a" in fp8 and not Z_BF16) else
                        blk.tile([1, QB], F32R, tag="zinv", bufs=1,
                                 name="zinv"))
                zb_sb = blk.tile([P, QB], F32, tag="zb", bufs=1, name="zb_sb")
                def emit_z_tail():
                    if "a" in fp8 and not Z_BF16:
                        # Z tail pair (c=7; c=0..6 were interleaved into the
                        # scores stream); psz rows are all Z (all-ones
                        # weights), so one reciprocal yields the broadcast
                        # 1/Z.
                        nc.tensor.matmul(psz, lhsT=ones8_sb,
                                         rhs=exp_sb[:, 14:16, :],
                                         start=False, stop=True, perf_mode=DR)
                        nc.vector.reciprocal(out=zb_sb, in_=psz)
                    else:
                        nc.tensor.matmul(psz[:1, :], lhsT=ones_sb[:, 0:1],
                                         rhs=acc, start=True, stop=True)
                        with nc.allow_low_precision(
                                reason="zinv feeds f32r matmul"):
                            nc.vector.reciprocal(out=zinv[:1, :],
                                                 in_=psz[:1, :])

                def emit_zb():
                    if "a" in fp8 and not Z_BF16:
                        return
                    psb = ps_sc.tile([P, QB], F32, tag="pss", bufs=2,
                                     name="psb")
                    nc.tensor.matmul(psb, lhsT=ones_sb[:1, :],
                                     rhs=zinv[:1, :], start=True, stop=True)
                    nc.vector.tensor_copy(out=zb_sb, in_=psb)

                # K-dense AV: one PSUM bank accumulates its full contraction
                # before the next starts (measured ~30ns/mm cheaper than
                # bank-interleaved issue).  The Z tail rides after the first
                # bank so the PE never waits on the last exp activation.
                for ft in range(2 if "a" in stages else 0):
                    psp = [ps_mid.tile([P, QB], F32, tag="psa", bufs=6,
                                       name=f"psa{j}") for j in range(4)]
                    for j in range(4):
                        for kc in range(8 if "a" in fp8 else KO):
                            if "a" in fp8:
                                nc.tensor.matmul(
                                    psp[j],
                                    lhsT=u_sb[:, 2 * kc:2 * kc + 2,
                                              ft * 512 + j * P:
                                              ft * 512 + (j + 1) * P],
                                    rhs=exp_sb[:, 2 * kc:2 * kc + 2, :],
                                    start=(kc == 0), stop=(kc == 7),
                                    perf_mode=DR)
                            else:
                                nc.tensor.matmul(
                                    psp[j],
                                    lhsT=u_sb[:, kc, ft * 512 + j * P:
                                              ft * 512 + (j + 1) * P],
                                    rhs=exp_sb[:, kc, :],
                                    start=(kc == 0), stop=(kc == KO - 1))
                        if ft == 0 and j == 0:
                            emit_z_tail()
                        if ft == 0 and j == 1:
                            emit_zb()
                    for j in range(4):
                        fo = ft * 4 + j
                        osa = blk.tile([P, QB], F32, tag="osa", bufs=2,
                                       name="osa")
                        nc.vector.tensor_mul(out=osa, in0=psp[j], in1=zb_sb)
                        ost = blk.tile([P, QB], F32, tag="ost", bufs=2,
                                       name="ost")
                        nc.scalar.activation(out=ost, in_=osa, func=AF.Identity,
                                             bias=bor_sb[:, fo:fo + 1],
                                             scale=1.0)
                        # stores ride the Activation HWDGE queue
                        nc.scalar.dma_start(
                            out=out_ap[fo * P:(fo + 1) * P, q0:q0 + QB],
                            in_=ost)

            blk.release()
            ps_sc.release()
            ps_mid.release()

        if loop_iters is None:
            body()
        else:
            with tc.For_i(0, loop_iters):
                body()

        persist.release()

    nc.compile()
    return nc


def _prep_shared(Wq, bq, Wk, bk, Wv, bv, Wo, bo, fp8=FP8_STAGES):
    def chunk_w(W, free, dt):
        wT = np.ascontiguousarray(np.asarray(W, dtype=np.float32).T)
        n = E // free
        return np.ascontiguousarray(
            wT.reshape(EO, P, n, free).transpose(2, 1, 0, 3).astype(dt))

    W64 = {k: np.asarray(v, dtype=np.float64)
           for k, v in dict(Wq=Wq, bq=bq, Wk=Wk, Wv=Wv, bv=bv, Wo=Wo,
                            bo=bo).items()}
    # Q.K^T and V.Wo^T weight fusions (see module docstring)
    M = (W64["Wk"].T @ W64["Wq"]).astype(np.float32)      # [e, e']
    G = (W64["Wo"] @ W64["Wv"]).astype(np.float32)        # [f, e']
    a2 = (W64["Wk"].T @ W64["bq"]).astype(np.float32)     # [e]
    bo_folded = (W64["bo"] + W64["Wo"] @ W64["bv"]).astype(np.float32)
    return {
        "mt": chunk_w(M * 32 if "d" in fp8 else M, P,
                      F8NP if "d" in fp8 else BFNP),
        "wv": chunk_w(G * 32 if "v" in fp8 else G, 512,
                      F8NP if "v" in fp8 else BFNP),
        "a2r": np.ascontiguousarray(a2.reshape(EO, P).T),
        "bor": np.ascontiguousarray(bo_folded.reshape(FO, P).T),
        "ones": np.ones((P, P), dtype=np.float32),
        "ones8": np.ones((P, 2, P), dtype=F8NP),
    }


def make_in_maps(x, Wq, bq, Wk, bk, Wv, bv, Wo, bo, fp8=FP8_STAGES):
    shared = _prep_shared(Wq, bq, Wk, bk, Wv, bv, Wo, bo, fp8)
    in_maps = []
    for c in range(N_CORES):
        b, h = c // 2, c % 2
        xt = np.asarray(x[b]).T  # [E, S]
        if h == 0:
            xt_p = np.ascontiguousarray(xt)
        else:
            xt_p = np.ascontiguousarray(
                np.concatenate([xt[:, SH:], xt[:, :SH]], axis=1))
        m = {"xt": xt_p.astype(BFNP), "xt8": xt_p.astype(F8NP)}
        m.update(shared)
        in_maps.append(m)
    return in_maps


def kernel(x, Wq, bq, Wk, bk, Wv, bv, Wo, bo):
    x = np.asarray(x, dtype=np.float32)
    args = [np.asarray(a, dtype=np.float32)
            for a in (Wq, bq, Wk, bk, Wv, bv, Wo, bo)]
    if "nc" not in _CACHE:
        _CACHE["nc"] = build_nc()
    nc = _CACHE["nc"]
    in_maps = make_in_maps(x, *args)
    res = bass_utils.run_bass_kernel_spmd(nc, in_maps,
                                          core_ids=list(range(N_CORES)))
    out = np.empty((B, S, E), dtype=np.float32)
    for c in range(N_CORES):
        b, h = c // 2, c % 2
        out[b, h * SH:(h + 1) * SH, :] = res.results[c]["out"].T
    return out
